# revision 1
# baseline (speedup 1.0000x reference)
"""DigitCaps (CapsNet dynamic routing) Trainium2 kernel.

Strategy: shard the I=1152 input capsules across the 8 cores (144 each).
Each core computes its u_hat shard [256, 10, 144, 16] with the tensor
engine (block-diagonal weight packing so K=32, M=128) and keeps it
resident in SBUF as bf16, laid out [p=batch(128), free=(i, o, d)] in two
batch chunks.  The three routing iterations then run on-chip; the only
cross-core data is the i-sum s [256, 10, 16], AllReduced per iteration
(softmax over o is pointwise in i, so everything else stays local).
"""

import numpy as np

B, O, I, DO, DI = 256, 10, 1152, 16, 8
N_CORES = 8
ISH = I // N_CORES          # 144 i's per core
NQ = ISH // 16              # 9 sixteen-i blocks (= xT blocks = quad groups)
BP = 128                    # batch partition chunk
NCH = B // BP               # 2 chunks
OD = O * DO                 # 160

_cached = {}


def _build():
    import concourse.mybir as mybir
    import concourse.tile as tile
    from concourse import bacc

    f32 = mybir.dt.float32
    bf16 = mybir.dt.bfloat16
    Alu = mybir.AluOpType
    Act = mybir.ActivationFunctionType
    X = mybir.AxisListType.X

    nc = bacc.Bacc("TRN2", target_bir_lowering=False, debug=False,
                   num_devices=N_CORES)

    # Per-core inputs (pre-arranged on host):
    # xT:  [NQ, 128, B]   rows = (i16, di8) for this 16-i block, cols = batch
    # Wbd: [NQ, 128, 640] 4 groups stacked; group sub's rows [32s,32s+32) hold
    #                     its block-diagonal [(j4,di8) x (j4,o,d)] weights
    # Wk:  [NQ, 128, 160] same stacking, dense [(j4,di8) x (o,d)] (for s1)
    xT_d = nc.dram_tensor("xT", [NQ, 128, B], bf16, kind="ExternalInput")
    Wbd_d = nc.dram_tensor("Wbd", [NQ, 128, 4 * OD], bf16, kind="ExternalInput")
    Wk_d = nc.dram_tensor("Wk", [NQ, 128, OD], bf16, kind="ExternalInput")
    y_d = nc.dram_tensor("y", [B, O, DO], f32, kind="ExternalOutput")

    with tile.TileContext(nc) as tc:
        with (
            tc.tile_pool(name="weights", bufs=1) as wpool,
            tc.tile_pool(name="uhat", bufs=1) as upool,
            tc.tile_pool(name="state", bufs=1) as stpool,
            tc.tile_pool(name="tmp", bufs=3) as tmppool,
            tc.tile_pool(name="small", bufs=2) as small,
            tc.tile_pool(name="psum_u", bufs=2, space="PSUM") as psum_u,
            tc.tile_pool(name="psum_s", bufs=2, space="PSUM") as psum_s,
            tc.tile_pool(name="dram", bufs=1, space="DRAM") as dram,
        ):
            # ---- load inputs ------------------------------------------------
            xT, Wbd, Wk = [], [], []
            for q in range(NQ):
                t = wpool.tile([128, B], bf16, name=f"xT{q}")
                nc.sync.dma_start(t[:], xT_d[q])
                xT.append(t)
                t = wpool.tile([128, 4 * OD], bf16, name=f"Wbd{q}")
                nc.sync.dma_start(t[:], Wbd_d[q])
                Wbd.append(t)
                t = wpool.tile([128, OD], bf16, name=f"Wk{q}")
                nc.sync.dma_start(t[:], Wk_d[q])
                Wk.append(t)

            # persistent per-chunk state
            u = [upool.tile([128, ISH, O, DO], bf16, name=f"u{ch}")
                 for ch in range(NCH)]
            bl = [stpool.tile([128, ISH, O], f32, name=f"b{ch}")
                  for ch in range(NCH)]
            cl = [stpool.tile([128, ISH, O], bf16, name=f"c{ch}")
                  for ch in range(NCH)]
            vb = [stpool.tile([128, O, DO], bf16, name=f"vb{ch}")
                  for ch in range(NCH)]

            ar_in = [dram.tile([NCH, 128, O, DO], f32, name=f"arin{t}")
                     for t in range(3)]
            ar_out = [dram.tile([NCH, 128, O, DO], f32, name=f"arout{t}")
                      for t in range(3)]

            # bank-aligned pieces of a duo psum [0,1280): (group, lo, hi)
            duo_pieces = [(0, 0, 512), (0, 512, 640),
                          (1, 640, 1024), (1, 1024, 1280)]

            # ---- phase 1: u_hat + s1 ---------------------------------------
            for ch in range(NCH):
                bsl = slice(ch * BP, ch * BP + BP)
                s1p = psum_s.tile([128, OD], f32, name="s1p")
                for q in range(NQ):
                    for duo in range(2):          # two 2-group duos per quad
                        dp = psum_u.tile([128, 1280], f32, name="dp")
                        for (gg, lo, hi) in duo_pieces:
                            sub = 2 * duo + gg    # group index within quad
                            lhsT = xT[q][32 * sub:32 * sub + 32, bsl]
                            nc.tensor.matmul(
                                dp[:, lo:hi],
                                lhsT,
                                Wbd[q][32 * sub:32 * sub + 32,
                                       lo - 640 * gg:hi - 640 * gg],
                                start=True, stop=True,
                                tile_position=(32 * sub, 0),
                            )
                        # evacuate duo -> u slice (8 i's, contiguous)
                        i0 = 16 * q + 8 * duo
                        dst = u[ch][:, i0:i0 + 8, :, :]
                        src = dp.rearrange("p (i o d) -> p i o d", i=8, o=O)
                        if (2 * q + duo) % 2 == 0:
                            nc.vector.tensor_copy(out=dst, in_=src)
                        else:
                            nc.scalar.copy(out=dst, in_=src)
                    # s1 partial: K=128 (sums the block's 16 i's — wanted),
                    # full-array matmul so the accumulation chain stays at a
                    # single tile position (mixed-position chains wedge HW).
                    nc.tensor.matmul(
                        s1p[:], xT[q][:, bsl], Wk[q][:],
                        start=(q == 0), stop=(q == NQ - 1),
                    )
                s1 = small.tile([128, O, DO], f32, name="s1")
                nc.scalar.mul(out=s1[:].rearrange("p o d -> p (o d)"),
                              in_=s1p[:], mul=0.1)
                nc.sync.dma_start(ar_in[0][ch], s1[:])

            # ---- helpers ----------------------------------------------------
            def squash(it, ch):
                """AllReduced s -> v (f32 in vb-bf16 + returns f32 tile)."""
                s = small.tile([128, O, DO], f32, name="ssum")
                nc.sync.dma_start(s[:], ar_out[it][ch])
                sq = small.tile([128, O, DO], f32, name="sq")
                nc.vector.tensor_mul(out=sq[:], in0=s[:], in1=s[:])
                n2 = small.tile([128, O], f32, name="n2")
                nc.vector.tensor_reduce(n2[:], sq[:], X, Alu.add)
                nrm = small.tile([128, O], f32, name="nrm")
                nc.scalar.activation(nrm[:], n2[:], Act.Sqrt)
                t1 = small.tile([128, O], f32, name="t1")
                nc.vector.tensor_scalar_add(t1[:], n2[:], 1.0)
                t2 = small.tile([128, O], f32, name="t2")
                nc.vector.tensor_scalar_add(t2[:], nrm[:], 1e-8)
                den = small.tile([128, O], f32, name="den")
                nc.vector.tensor_mul(out=den[:], in0=t1[:], in1=t2[:])
                rden = small.tile([128, O], f32, name="rden")
                nc.vector.reciprocal(out=rden[:], in_=den[:])
                scl = small.tile([128, O], f32, name="scl")
                nc.vector.tensor_mul(out=scl[:], in0=n2[:], in1=rden[:])
                vf = small.tile([128, O, DO], f32, name="vf")
                nc.vector.tensor_tensor(
                    vf[:], s[:],
                    scl[:, :, None].to_broadcast([128, O, DO]), Alu.mult)
                nc.vector.tensor_copy(out=vb[ch][:], in_=vf[:])
                return vf

            def uv_pass(ch, first):
                """b += u . v  (contract over d); first iter writes b."""
                for q in range(NQ):
                    isl = slice(16 * q, 16 * q + 16)
                    tmp = tmppool.tile([128, 16, O, DO], f32, name="uvtmp")
                    nc.vector.tensor_tensor(
                        tmp[:], u[ch][:, isl, :, :],
                        vb[ch][:, None, :, :].to_broadcast([128, 16, O, DO]),
                        Alu.mult)
                    if first:
                        nc.vector.tensor_reduce(
                            bl[ch][:, isl, :], tmp[:], X, Alu.add)
                    else:
                        uvb = small.tile([128, 16, O], f32, name="uvb")
                        nc.vector.tensor_reduce(uvb[:], tmp[:], X, Alu.add)
                        nc.vector.tensor_add(
                            out=bl[ch][:, isl, :],
                            in0=bl[ch][:, isl, :], in1=uvb[:])

            def c_pass(ch):
                """c = softmax_o(b)   (b is small enough to be exp-safe)."""
                e = small.tile([128, ISH, O], bf16, name="e")
                nc.scalar.activation(e[:], bl[ch][:], Act.Exp)
                Z = small.tile([128, ISH], f32, name="Z")
                nc.vector.tensor_reduce(Z[:], e[:], X, Alu.add)
                rZ = small.tile([128, ISH], bf16, name="rZ")
                with nc.allow_low_precision(reason="softmax denom, |b| << 1"):
                    nc.vector.reciprocal(out=rZ[:], in_=Z[:])
                nc.vector.tensor_tensor(
                    cl[ch][:], e[:],
                    rZ[:, :, None].to_broadcast([128, ISH, O]), Alu.mult)

            def s_pass(it, ch):
                """s_partial = sum_i c*u -> ar_in[it][ch]."""
                sacc = small.tile([128, O, DO], f32, name="sacc")
                for q in range(NQ):
                    isl = slice(16 * q, 16 * q + 16)
                    tmp2 = tmppool.tile([128, O, DO, 16], f32, name="stmp")
                    nc.vector.tensor_tensor(
                        tmp2.rearrange("p o d i -> p i o d"),
                        u[ch][:, isl, :, :],
                        cl[ch][:, isl, :, None].to_broadcast(
                            [128, 16, O, DO]),
                        Alu.mult)
                    if q == 0:
                        nc.vector.tensor_reduce(sacc[:], tmp2[:], X, Alu.add)
                    else:
                        sblk = small.tile([128, O, DO], f32, name="sblk")
                        nc.vector.tensor_reduce(sblk[:], tmp2[:], X, Alu.add)
                        nc.vector.tensor_add(out=sacc[:], in0=sacc[:],
                                             in1=sblk[:])
                nc.sync.dma_start(ar_in[it][ch], sacc[:])

            def allreduce(it):
                nc.gpsimd.collective_compute(
                    "AllReduce", Alu.add,
                    replica_groups=[list(range(N_CORES))],
                    ins=[ar_in[it].opt()], outs=[ar_out[it].opt()])

            # ---- routing ----------------------------------------------------
            allreduce(0)                      # iteration 1: s1
            for ch in range(NCH):
                squash(0, ch)                 # v1
                uv_pass(ch, first=True)       # b = u.v1

            for ch in range(NCH):             # iteration 2
                c_pass(ch)
                s_pass(1, ch)
            allreduce(1)
            for ch in range(NCH):
                squash(1, ch)                 # v2
                uv_pass(ch, first=False)      # b += u.v2

            for ch in range(NCH):             # iteration 3 (final)
                c_pass(ch)
                s_pass(2, ch)
            allreduce(2)
            for ch in range(NCH):
                vf = squash(2, ch)            # v3 = output
                nc.sync.dma_start(y_d[ch * BP:ch * BP + BP, :, :], vf[:])

    nc.compile()
    return nc


def _prep_inputs(x, weight):
    """Host-side resharding: returns per-core input dicts."""
    import ml_dtypes

    bf = ml_dtypes.bfloat16
    x = np.asarray(x, dtype=np.float32)
    w = np.asarray(weight, dtype=np.float32)[0]      # [O, I, DO, DI]
    in_maps = []
    for c in range(N_CORES):
        isl = slice(c * ISH, (c + 1) * ISH)
        xs = x[:, isl, :]                            # [B, 144, 8]
        # xT[q, (si16, di8), b]
        xT = xs.reshape(B, NQ, 16, DI).transpose(1, 2, 3, 0).reshape(
            NQ, 128, B)
        ws = w[:, isl, :, :]                         # [O, 144, DO, DI]
        # Wg[g, j, di, (o d)]
        wg = ws.transpose(1, 3, 0, 2).reshape(ISH // 4, 4, DI, OD)
        wk = wg.reshape(ISH // 4, 32, OD)            # dense K=32 blocks
        wbd = np.zeros((ISH // 4, 32, 4 * OD), dtype=np.float32)
        for j in range(4):
            wbd[:, 8 * j:8 * j + 8, OD * j:OD * j + OD] = wg[:, j]
        in_maps.append({
            "xT": np.ascontiguousarray(xT.astype(bf)),
            "Wbd": np.ascontiguousarray(
                wbd.reshape(NQ, 128, 4 * OD).astype(bf)),
            "Wk": np.ascontiguousarray(wk.reshape(NQ, 128, OD).astype(bf)),
        })
    return in_maps


def kernel(x, weight):
    from concourse.bass_utils import run_bass_kernel_spmd

    if "nc" not in _cached:
        _cached["nc"] = _build()
    in_maps = _prep_inputs(x, weight)
    res = run_bass_kernel_spmd(
        _cached["nc"], in_maps, core_ids=list(range(N_CORES)))
    return res.results[0]["y"].astype(np.float32)



# revision 10
# speedup vs baseline: 40.6773x; 40.6773x over previous
"""DigitCaps (CapsNet dynamic routing) Trainium2 kernel — batch-parallel.

Strategy: pure data parallelism over batch (32 of 256 per core), zero
collectives, so the 8 cores never synchronize.  Per core:

  u_hat[b,o,i,d] is produced by 288 "quad" matmuls: for each group of 4
  input capsules the block-diagonalized x slice ([K=32=(i4,k8), M=128=
  (i4,b32)]) is the stationary operand and the dense weight slice
  ([32, 160=(o,d)]) streams, giving PSUM [p=(i_off,b), (o,d)].  u lives
  in SBUF as bf16 [p=(i_off,b), (o, j=288 quads, d)].

  Iteration 1's softmax is uniform (b=0 -> c=0.1), so s1 = 0.1*sum_i u
  comes from a dense 72-matmul accumulation chain contracting (i,k)
  directly — no routing pass needed.

  Routing iterations then run on-chip: softmax over o (ACT exp + DVE),
  s = sum_i c*u via elementwise mult (DVE/Pool) + a PE accumulation
  chain against a constant selection matrix (fold of the 4 i_off
  partitions and the 288 j's into [b=32, (o,d)]), squash on 32
  partitions, v replicated 32->128 partitions by a tiny matmul, and
  uv = sum_d u*v on DVE (2x mode: all-bf16, unit innermost strides).
"""

import numpy as np

B, O, I, DO, DI = 256, 10, 1152, 16, 8
N_CORES = 8
BC = B // N_CORES           # 32 batches per core
G = 72                      # 16-i groups (s1 chain length)
NQ = I // 4                 # 288 quads (4 i's each)
OD = O * DO                 # 160
JC = 36                     # j-chunk for routing passes
NCHK = NQ // JC             # 8 chunks

# blob column offsets (bf16 elements), one [128, BLOB] input per core
XT_OFF = 0
XT_SZ = G * BC              # 2304
W_OFF = XT_OFF + XT_SZ
W_SZ = G * OD               # 11520
XBD_OFF = W_OFF + W_SZ
XBD_SZ = G * 128            # 9216
SEL_OFF = XBD_OFF + XBD_SZ  # 32 cols
REP_OFF = SEL_OFF + 32      # 128 cols
BLOB = REP_OFF + 128

_cached = {}


def _build(repeat=1, use_loop=False):
    import concourse.mybir as mybir
    import concourse.tile as tile
    from concourse import bacc

    f32 = mybir.dt.float32
    bf16 = mybir.dt.bfloat16
    Alu = mybir.AluOpType
    Act = mybir.ActivationFunctionType
    X = mybir.AxisListType.X

    nc = bacc.Bacc("TRN2", target_bir_lowering=False, debug=False,
                   num_devices=N_CORES)

    blob_d = nc.dram_tensor("blob", [128, BLOB], bf16, kind="ExternalInput")
    y_d = nc.dram_tensor("y", [BC, O, DO], f32, kind="ExternalOutput")

    with tile.TileContext(nc) as tc:
        with (
            tc.tile_pool(name="inp", bufs=1) as inpool,
            tc.tile_pool(name="uhat", bufs=1) as upool,
            tc.tile_pool(name="state", bufs=1) as stpool,
            tc.tile_pool(name="tmp", bufs=3) as tmppool,
            tc.tile_pool(name="small", bufs=2) as small,
            tc.tile_pool(name="yout", bufs=1) as ypool,
            tc.tile_pool(name="psum_u", bufs=3, space="PSUM") as pu,
            tc.tile_pool(name="psum_s1", bufs=1, space="PSUM") as ps1,
            tc.tile_pool(name="psum_s", bufs=2, space="PSUM") as ps,
            tc.tile_pool(name="psum_v", bufs=2, space="PSUM") as pv,
        ):
            big = inpool.tile([128, BLOB], bf16, name="big")
            nc.sync.dma_start(big[:], blob_d[:])
            xt = big[:, XT_OFF:XT_OFF + XT_SZ].rearrange(
                "p (g b) -> p g b", g=G)
            Wt = big[:, W_OFF:W_OFF + W_SZ].rearrange(
                "p (g n) -> p g n", g=G)
            xbd = big[:, XBD_OFF:XBD_OFF + XBD_SZ].rearrange(
                "p (g m) -> p g m", g=G)
            sel = big[:, SEL_OFF:SEL_OFF + 32]
            rep = big[0:32, REP_OFF:REP_OFF + 128]

            # persistent state
            u = upool.tile([128, O, NQ, DO], bf16, name="u")
            bl = stpool.tile([128, O, NQ], bf16, name="bl")
            e = stpool.tile([128, O, NQ], bf16, name="e")
            cl = stpool.tile([128, O, NQ], bf16, name="cl")
            vrep = stpool.tile([128, O, DO], bf16, name="vrep")
            vf_out = ypool.tile([32, O, DO], f32, name="vf_out")

            def squash(s_ap, vf=None):
                """s [32, O, DO] (f32, may be PSUM) -> vb bf16 [32, OD] for
                the replicate matmul; optionally also writes vf (f32)."""
                sq = small.tile([32, O, DO], f32, name="sq")
                nc.scalar.square(out=sq[:], in_=s_ap)
                n2 = small.tile([32, O], f32, name="n2")
                nc.vector.tensor_reduce(n2[:], sq[:], X, Alu.add)
                nrm = small.tile([32, O], f32, name="nrm")
                nc.scalar.activation(nrm[:], n2[:], Act.Sqrt)
                t1 = small.tile([32, O], f32, name="t1")
                nc.vector.tensor_scalar_add(t1[:], n2[:], 1.0)
                t2 = small.tile([32, O], f32, name="t2")
                nc.vector.tensor_scalar_add(t2[:], nrm[:], 1e-8)
                den = small.tile([32, O], f32, name="den")
                nc.vector.tensor_mul(out=den[:], in0=t1[:], in1=t2[:])
                rden = small.tile([32, O], f32, name="rden")
                nc.vector.reciprocal(out=rden[:], in_=den[:])
                scl = small.tile([32, O], f32, name="scl")
                nc.vector.tensor_mul(out=scl[:], in0=n2[:], in1=rden[:])
                vb = small.tile([32, O, DO], bf16, name="vb")
                nc.vector.tensor_tensor(
                    vb[:], s_ap,
                    scl[:, :, None].to_broadcast([32, O, DO]), Alu.mult)
                if vf is not None:
                    nc.vector.tensor_tensor(
                        vf, s_ap,
                        scl[:, :, None].to_broadcast([32, O, DO]), Alu.mult)
                return vb

            def replicate(vb):
                """vb [32, O, DO] bf16 -> vrep [128, O, DO] bf16."""
                pvt = pv.tile([128, 512], f32, name="pvt")[:, 0:OD]
                nc.tensor.matmul(pvt, rep, vb[:].rearrange(
                    "p o d -> p (o d)"), start=True, stop=True)
                nc.scalar.copy(
                    out=vrep[:],
                    in_=pvt.rearrange("p (o d) -> p o d", o=O))

            def uv_chunk(ci, first):
                """bl[:, :, chunk] (+)= sum_d u*vrep for one j-chunk."""
                j0 = ci * JC
                pr = tmppool.tile([128, O, JC, DO], bf16, name="pr")
                nc.vector.tensor_tensor(
                    pr[:], u[:, :, j0:j0 + JC, :],
                    vrep[:, :, None, :].to_broadcast([128, O, JC, DO]),
                    Alu.mult)
                with nc.allow_low_precision(reason="16-term uv sum, |b|<<1"):
                    if first:
                        nc.vector.tensor_reduce(
                            bl[:, :, j0:j0 + JC], pr[:], X, Alu.add)
                    else:
                        uvc = small.tile([128, O, JC], bf16, name="uvc")
                        nc.vector.tensor_reduce(uvc[:], pr[:], X, Alu.add)
                        nc.vector.tensor_add(
                            out=bl[:, :, j0:j0 + JC],
                            in0=bl[:, :, j0:j0 + JC], in1=uvc[:])

            def c_pass():
                nc.scalar.activation(e[:], bl[:], Act.Exp)
                Z = small.tile([128, NQ], f32, name="Z")
                nc.vector.tensor_reduce(
                    Z[:], e[:].rearrange("p o j -> p j o"), X, Alu.add)
                rZ = small.tile([128, NQ], bf16, name="rZ")
                with nc.allow_low_precision(reason="softmax denom ~10"):
                    nc.vector.reciprocal(out=rZ[:], in_=Z[:])
                nc.vector.tensor_tensor(
                    cl[:], e[:],
                    rZ[:, None, :].to_broadcast([128, O, NQ]), Alu.mult)

            def s_pass():
                """sum_i c*u -> PSUM [32, OD] via mult + PE fold chain."""
                psx = ps.tile([32, 512], f32, name="psx")[:, 0:OD]
                for ci in range(NCHK):
                    j0 = ci * JC
                    pr = tmppool.tile([128, O, JC, DO], bf16, name="pr")
                    eng = nc.vector
                    eng.tensor_tensor(
                        pr[:], u[:, :, j0:j0 + JC, :],
                        cl[:, :, j0:j0 + JC, None].to_broadcast(
                            [128, O, JC, DO]),
                        Alu.mult)
                    for jj in range(JC):
                        j = j0 + jj
                        nc.tensor.matmul(
                            psx, sel, pr[:, :, jj, :],
                            start=(j == 0), stop=(j == NQ - 1))
                return psx

            def body():
                # s1 chain first so v1/vrep1 are ready early
                s1p = ps1.tile([32, 512], f32, name="s1p")[:, 0:OD]
                for g in range(G):
                    nc.tensor.matmul(s1p, xt[:, g, :], Wt[:, g, :],
                                     start=(g == 0), stop=(g == G - 1))
                s1f = small.tile([32, O, DO], f32, name="s1f")
                nc.scalar.mul(out=s1f[:].rearrange("p o d -> p (o d)"),
                              in_=s1p, mul=0.1)
                replicate(squash(s1f[:]))

                # u_hat quads.  All 3 matmuls of a psum tile share one
                # tile_position (mixing positions within one PSUM tile
                # wedges the PE); s rotates tile-to-tile so uv1 chunks can
                # interleave on DVE as u fills.  j = 4*g + s.
                uj = u.rearrange("p o (g s) d -> p o g s d", s=4)
                done = [0] * 4            # per-s completed g count
                uv_done = 0
                for t in range(96):
                    tt, s = divmod(t, 4)
                    pt = pu.tile([128, 512], f32, name="pt")
                    for q3 in range(3):
                        g = 3 * tt + q3
                        nc.tensor.matmul(
                            pt[:, 160 * q3:160 * (q3 + 1)],
                            xbd[32 * s:32 * s + 32, g, :],
                            Wt[32 * s:32 * s + 32, g, :],
                            start=True, stop=True,
                            tile_position=(32 * s, 0))
                    dst = uj[:, :, 3 * tt:3 * tt + 3, s, :]
                    src = pt[:, 0:480].rearrange(
                        "p (q o d) -> p o q d", q=3, o=O)
                    if t % 2 == 0:
                        nc.vector.tensor_copy(out=dst, in_=src)
                    else:
                        nc.scalar.copy(out=dst, in_=src)
                    done[s] = 3 * tt + 3
                    # uv chunk ci needs j in [ci*JC, (ci+1)*JC) = g range
                    # [ci*9, ci*9+9) complete at every s
                    while uv_done < NCHK and min(done) >= (uv_done + 1) * 9:
                        uv_chunk(uv_done, first=True)
                        uv_done += 1

                # iteration 2
                c_pass()
                vb2 = squash(s_pass().rearrange("p (o d) -> p o d", o=O))
                replicate(vb2)
                for ci in range(NCHK):
                    uv_chunk(ci, first=False)

                # iteration 3 (final)
                c_pass()
                squash(s_pass().rearrange("p (o d) -> p o d", o=O),
                       vf=vf_out[:])

            if use_loop or repeat > 1:
                with tc.For_i(0, repeat):
                    body()
            else:
                body()

            nc.sync.dma_start(y_d[:], vf_out[:])

    nc.compile()
    return nc


def _prep_inputs(x, weight):
    """Host-side packing: one [128, BLOB] bf16 blob per core."""
    import ml_dtypes

    bf = ml_dtypes.bfloat16
    x = np.asarray(x, dtype=np.float32)
    w0 = np.asarray(weight, dtype=np.float32)[0]        # [O, I, DO, DI]

    # weights, dense per quad: W[(s,i,k) part, g, (o,d)]
    wp = w0.reshape(O, G, 4, 4, DO, DI).transpose(1, 2, 3, 5, 0, 4)
    wp = wp.reshape(G, 128, OD).transpose(1, 0, 2).reshape(128, W_SZ)
    wp = np.ascontiguousarray(wp.astype(bf))

    selm = np.zeros((128, 32), dtype=np.float32)
    for i in range(4):
        selm[32 * i:32 * i + 32] = np.eye(32, dtype=np.float32)
    repm = np.zeros((128, 128), dtype=np.float32)
    for i in range(4):
        repm[0:32, 32 * i:32 * i + 32] = np.eye(32, dtype=np.float32)
    selm = selm.astype(bf)
    repm = repm.astype(bf)

    in_maps = []
    for c in range(N_CORES):
        xs = x[BC * c:BC * (c + 1)]                     # [32, 1152, 8]
        xr = xs.reshape(BC, G, 4, 4, DI).transpose(1, 2, 3, 4, 0)
        # xr: [g, s, i, k, b]
        xtp = xr.reshape(G, 128, BC).transpose(1, 0, 2).reshape(128, XT_SZ)
        xbd6 = np.zeros((G, 4, 4, DI, 4, BC), dtype=np.float32)
        for i in range(4):
            xbd6[:, :, i, :, i, :] = xr[:, :, i, :, :]
        xbdp = xbd6.reshape(G, 128, 128).transpose(1, 0, 2).reshape(
            128, XBD_SZ)
        blob = np.empty((128, BLOB), dtype=bf)
        blob[:, XT_OFF:XT_OFF + XT_SZ] = xtp.astype(bf)
        blob[:, W_OFF:W_OFF + W_SZ] = wp
        blob[:, XBD_OFF:XBD_OFF + XBD_SZ] = xbdp.astype(bf)
        blob[:, SEL_OFF:SEL_OFF + 32] = selm
        blob[:, REP_OFF:REP_OFF + 128] = repm
        in_maps.append({"blob": blob})
    return in_maps


def kernel(x, weight):
    from concourse.bass_utils import run_bass_kernel_spmd

    if "nc" not in _cached:
        _cached["nc"] = _build()
    in_maps = _prep_inputs(x, weight)
    res = run_bass_kernel_spmd(
        _cached["nc"], in_maps, core_ids=list(range(N_CORES)))
    return np.concatenate(
        [res.results[c]["y"] for c in range(N_CORES)], axis=0
    ).astype(np.float32)


# revision 22
# speedup vs baseline: 108.2100x; 2.6602x over previous
"""DigitCaps (CapsNet dynamic routing) Trainium2 kernel — batch-parallel.

Strategy: pure data parallelism over batch (32 of 256 per core), zero
collectives, so the 8 cores never synchronize.  Per core:

  u_hat[b,o,i,d] is produced by 288 "quad" matmuls: for each group of 4
  input capsules the block-diagonalized x slice ([K=32=(i4,k8), M=128=
  (i4,b32)]) is the stationary operand and the dense weight slice
  ([32, 160=(o,d)]) streams, giving PSUM [p=(i_off,b), (o,d)].  u lives
  in SBUF as bf16 [p=(i_off,b), (o, j=288 quads, d)].

  Iteration 1's softmax is uniform (b=0 -> c=0.1), so s1 = 0.1*sum_i u
  comes from a dense 72-matmul accumulation chain contracting (i,k)
  directly — no routing pass needed.

  Routing iterations then run on-chip: softmax over o (ACT exp + DVE),
  s = sum_i c*u via elementwise mult (split DVE/Pool) + a PE
  accumulation chain against a constant selection matrix (fold of the
  4 i_off partitions and the 288 j's into [b=32, (o,d)]), squash on 32
  partitions, v replicated 32->128 partitions by a tiny matmul, and
  uv = sum_d u*v (Pool mult + DVE reduce).  All big operands bf16 with
  contiguous writes — DVE strided writes measured ~3x slower.
"""

import numpy as np

B, O, I, DO, DI = 256, 10, 1152, 16, 8
N_CORES = 8
BC = B // N_CORES           # 32 batches per core
G = 72                      # 16-i groups (s1 chain length)
NQ = I // 4                 # 288 quads (4 i's each)
OD = O * DO                 # 160
JC = 36                     # j-chunk for routing passes
NCHK = NQ // JC             # 8 chunks

# blob column offsets (bf16 elements), one [128, BLOB] input per core
XT_OFF = 0
XT_SZ = G * BC              # 2304
W_OFF = XT_OFF + XT_SZ
W_SZ = G * OD               # 11520
XBD_SZ = G * 128            # 9216 (built on device from xt)
SEL_OFF = W_OFF + W_SZ      # 32 cols
REP_OFF = SEL_OFF + 32      # 128 cols
BLOB = REP_OFF + 128

_cached = {}


def _build(repeat=1, use_loop=False, level=5):
    import concourse.mybir as mybir
    import concourse.tile as tile
    from concourse import bacc

    f32 = mybir.dt.float32
    bf16 = mybir.dt.bfloat16
    Alu = mybir.AluOpType
    Act = mybir.ActivationFunctionType
    X = mybir.AxisListType.X

    nc = bacc.Bacc("TRN2", target_bir_lowering=False, debug=False,
                   num_devices=N_CORES)

    blob_d = nc.dram_tensor("blob", [128, BLOB], bf16, kind="ExternalInput")
    y_d = nc.dram_tensor("y", [BC, O, DO], f32, kind="ExternalOutput")

    with tile.TileContext(nc) as tc:
        with (
            tc.tile_pool(name="inp", bufs=1) as inpool,
            tc.tile_pool(name="uhat", bufs=1) as upool,
            tc.tile_pool(name="state", bufs=1) as stpool,
            tc.tile_pool(name="tmp", bufs=3) as tmppool,
            tc.tile_pool(name="small", bufs=2) as small,
            tc.tile_pool(name="yout", bufs=1) as ypool,
            tc.tile_pool(name="psum_u", bufs=3, space="PSUM") as pu,
            tc.tile_pool(name="psum_s1", bufs=1, space="PSUM") as ps1,
            tc.tile_pool(name="psum_s", bufs=2, space="PSUM") as ps,
            tc.tile_pool(name="psum_v", bufs=2, space="PSUM") as pv,
        ):
            big = inpool.tile([128, BLOB], bf16, name="big")
            nc.sync.dma_start(big[:], blob_d[:])
            xt = big[:, XT_OFF:XT_OFF + XT_SZ].rearrange(
                "p (g b) -> p g b", g=G)
            Wt = big[:, W_OFF:W_OFF + W_SZ].rearrange(
                "p (g n) -> p g n", g=G)
            sel = big[:, SEL_OFF:SEL_OFF + 32]
            rep = big[0:32, REP_OFF:REP_OFF + 128]

            # block-diagonalized x, built on device: xbd[32s+8i'+k, g,
            # 32i+b] = xt[32s+8i+k, g, b] iff i'==i, else 0
            xbd_t = inpool.tile([128, G, 128], bf16, name="xbd")
            xbd = xbd_t[:]
            nc.gpsimd.memset(xbd_t[:], 0.0)
            for s4 in range(4):
                for i4 in range(4):
                    r0 = 32 * s4 + 8 * i4
                    nc.sync.dma_start(
                        xbd_t[r0:r0 + 8, :, 32 * i4:32 * i4 + 32],
                        xt[r0:r0 + 8, :, :])

            # persistent state
            u = upool.tile([128, O, NQ, DO], bf16, name="u")
            bl = stpool.tile([128, O, NQ], bf16, name="bl")
            e = stpool.tile([128, O, NQ], bf16, name="e")
            cl = stpool.tile([128, O, NQ], bf16, name="cl")
            vrep = stpool.tile([128, O, DO], bf16, name="vrep")
            vf_out = ypool.tile([32, O, DO], f32, name="vf_out")

            def squash(s_ap, vf=None):
                """s [32, O, DO] (f32, may be PSUM) -> vb bf16 [32, OD] for
                the replicate matmul; optionally also writes vf (f32)."""
                sq = small.tile([32, O, DO], f32, name="sq")
                nc.scalar.square(out=sq[:], in_=s_ap)
                n2 = small.tile([32, O], f32, name="n2")
                nc.vector.tensor_reduce(n2[:], sq[:], X, Alu.add)
                nrm = small.tile([32, O], f32, name="nrm")
                nc.scalar.activation(nrm[:], n2[:], Act.Sqrt)
                t1 = small.tile([32, O], f32, name="t1")
                nc.vector.tensor_scalar_add(t1[:], n2[:], 1.0)
                t2 = small.tile([32, O], f32, name="t2")
                nc.vector.tensor_scalar_add(t2[:], nrm[:], 1e-8)
                den = small.tile([32, O], f32, name="den")
                nc.vector.tensor_mul(out=den[:], in0=t1[:], in1=t2[:])
                rden = small.tile([32, O], f32, name="rden")
                nc.vector.reciprocal(out=rden[:], in_=den[:])
                scl = small.tile([32, O], f32, name="scl")
                nc.vector.tensor_mul(out=scl[:], in0=n2[:], in1=rden[:])
                vb = small.tile([32, O, DO], bf16, name="vb")
                nc.vector.tensor_tensor(
                    vb[:], s_ap,
                    scl[:, :, None].to_broadcast([32, O, DO]), Alu.mult)
                if vf is not None:
                    nc.vector.tensor_tensor(
                        vf, s_ap,
                        scl[:, :, None].to_broadcast([32, O, DO]), Alu.mult)
                return vb

            def replicate(vb):
                """vb [32, O, DO] bf16 -> vrep [128, O, DO] bf16."""
                pvt = pv.tile([128, 512], f32, name="pvt")[:, 0:OD]
                nc.tensor.matmul(pvt, rep, vb[:].rearrange(
                    "p o d -> p (o d)"), start=True, stop=True)
                nc.scalar.copy(
                    out=vrep[:],
                    in_=pvt.rearrange("p (o d) -> p o d", o=O))

            def uv_chunk(ci):
                """bl[:, :, chunk] = sum_d u*vrep for one j-chunk.  uv is
                linear in v, so round B feeds v1+v2 instead of
                accumulating: bl is always a fresh write.  Pool does the
                mult (same rate as DVE here), DVE the d-reduce."""
                j0 = ci * JC
                pr = tmppool.tile([128, O, JC, DO], bf16, name="pr")
                ueng = nc.vector if ci == 7 else nc.gpsimd
                ueng.tensor_tensor(
                    pr[:], u[:, :, j0:j0 + JC, :],
                    vrep[:, :, None, :].to_broadcast([128, O, JC, DO]),
                    Alu.mult)
                with nc.allow_low_precision(reason="16-term uv sum, |b|<<1"):
                    nc.vector.tensor_reduce(
                        bl[:, :, j0:j0 + JC], pr[:], X, Alu.add)

            def c_pass():
                nc.scalar.activation(e[:], bl[:], Act.Exp)
                Z = small.tile([128, NQ], f32, name="Z")
                nc.vector.tensor_reduce(
                    Z[:], e[:].rearrange("p o j -> p j o"), X, Alu.add)
                rZ = small.tile([128, NQ], bf16, name="rZ")
                with nc.allow_low_precision(reason="softmax denom ~10"):
                    nc.vector.reciprocal(out=rZ[:], in_=Z[:])
                nc.vector.tensor_tensor(
                    cl[:], e[:],
                    rZ[:, None, :].to_broadcast([128, O, NQ]), Alu.mult)

            def s_pass():
                """sum_i c*u -> PSUM [32, OD].  Contiguous-write mults
                split Pool/DVE; the otherwise-idle PE folds both j and
                the 4 i_off partitions via an accumulation chain against
                the constant selection matrix (strided writes on DVE cost
                ~3x, so no on-DVE j-reduce)."""
                psx = ps.tile([32, 512], f32, name="psx")[:, 0:OD]
                for ci in range(NCHK):
                    j0 = ci * JC
                    pr = tmppool.tile([128, O, JC, DO], bf16, name="pr")
                    eng = nc.gpsimd if ci % 2 == 0 else nc.vector
                    eng.tensor_tensor(
                        pr[:], u[:, :, j0:j0 + JC, :],
                        cl[:, :, j0:j0 + JC, None].to_broadcast(
                            [128, O, JC, DO]),
                        Alu.mult)
                    for jj in range(JC):
                        j = j0 + jj
                        nc.tensor.matmul(
                            psx, sel, pr[:, :, jj, :],
                            start=(j == 0), stop=(j == NQ - 1))
                return psx

            def body():
                if level == 0:
                    nc.vector.memset(vf_out[:], 0.0)
                    return
                # s1 chain first so v1/vrep1 are ready early
                s1p = ps1.tile([32, 512], f32, name="s1p")[:, 0:OD]
                for g in range(G):
                    nc.tensor.matmul(s1p, xt[:, g, :], Wt[:, g, :],
                                     start=(g == 0), stop=(g == G - 1))
                s1f = small.tile([32, O, DO], f32, name="s1f")
                nc.scalar.mul(out=s1f[:].rearrange("p o d -> p (o d)"),
                              in_=s1p, mul=0.1)
                vf1 = stpool.tile([32, O, DO], f32, name="vf1")
                replicate(squash(s1f[:], vf=vf1[:]))
                if level != 5:
                    nc.scalar.copy(out=vf_out[:], in_=vf1[:])

                # u_hat quads.  All 3 matmuls of a psum tile share one
                # tile_position (mixing positions within one PSUM tile
                # wedges the PE); s rotates tile-to-tile so uv1 chunks can
                # interleave on DVE as u fills.  j = 4*g + s.
                uj = u.rearrange("p o (g s) d -> p o g s d", s=4)
                done = [0] * 4            # per-s completed g count
                uv_done = 0
                for t in range(96):
                    tt, s = divmod(t, 4)
                    pt = pu.tile([128, 512], f32, name="pt")
                    for q3 in range(3):
                        g = 3 * tt + q3
                        nc.tensor.matmul(
                            pt[:, 160 * q3:160 * (q3 + 1)],
                            xbd[32 * s:32 * s + 32, g, :],
                            Wt[32 * s:32 * s + 32, g, :],
                            start=True, stop=True,
                            tile_position=(32 * s, 0))
                    dst = uj[:, :, 3 * tt:3 * tt + 3, s, :]
                    src = pt[:, 0:480].rearrange(
                        "p (q o d) -> p o q d", q=3, o=O)
                    nc.scalar.copy(out=dst, in_=src)
                    done[s] = 3 * tt + 3
                    # uv chunk ci needs j in [ci*JC, (ci+1)*JC) = g range
                    # [ci*9, ci*9+9) complete at every s
                    while (level >= 2 and uv_done < NCHK
                           and min(done) >= (uv_done + 1) * 9):
                        uv_chunk(uv_done)
                        uv_done += 1

                if level in (11, 12, 13, 14, 15, 16):
                    nc.vector.memset(cl[:], 0.0)
                    for ci in range(NCHK):
                        j0 = ci * JC
                        if level == 11:
                            pr = tmppool.tile([128, O, JC, DO], bf16,
                                              name="pr")
                            nc.vector.tensor_tensor(
                                pr[:], u[:, :, j0:j0 + JC, :],
                                vrep[:, :, None, :].to_broadcast(
                                    [128, O, JC, DO]), Alu.mult)
                        elif level == 12:
                            pr = tmppool.tile([128, O, DO, JC], bf16,
                                              name="pr")
                            nc.vector.tensor_tensor(
                                pr[:].rearrange("p o d j -> p o j d"),
                                u[:, :, j0:j0 + JC, :],
                                cl[:, :, j0:j0 + JC, None].to_broadcast(
                                    [128, O, JC, DO]), Alu.mult)
                        elif level == 13:
                            pr = tmppool.tile([128, O, JC, DO], bf16,
                                              name="pr")
                            nc.vector.tensor_tensor(
                                pr[:], u[:, :, j0:j0 + JC, :],
                                cl[:, :, j0:j0 + JC, None].to_broadcast(
                                    [128, O, JC, DO]), Alu.mult)
                        elif level == 14:
                            pr = tmppool.tile([128, O, JC, DO], bf16,
                                              name="pr")
                            nc.vector.tensor_tensor(
                                pr[:], u[:, :, j0:j0 + JC, :],
                                u[:, :, j0:j0 + JC, :], Alu.mult)
                        elif level == 15:
                            pr = tmppool.tile([128, O, JC, DO], bf16,
                                              name="pr")
                            nc.vector.tensor_copy(
                                out=pr[:], in_=u[:, :, j0:j0 + JC, :])
                        else:
                            uvc = small.tile([128, O, JC], bf16,
                                             name="uvc")
                            with nc.allow_low_precision(reason="bench"):
                                nc.vector.tensor_reduce(
                                    uvc[:], u[:, :, j0:j0 + JC, :], X,
                                    Alu.add)
                    return

                # iteration 2
                if level >= 3:
                    c_pass()
                    vf2 = small.tile([32, O, DO], f32, name="vf2")
                    squash(s_pass().rearrange("p (o d) -> p o d", o=O),
                           vf=vf2[:])
                    vs = small.tile([32, O, DO], bf16, name="vs")
                    nc.vector.tensor_add(out=vs[:], in0=vf1[:],
                                         in1=vf2[:])
                    replicate(vs)
                if level >= 4:
                    for ci in range(NCHK):
                        uv_chunk(ci)

                # iteration 3 (final)
                if level >= 5:
                    c_pass()
                    squash(s_pass().rearrange("p (o d) -> p o d", o=O),
                           vf=vf_out[:])

            if use_loop or repeat > 1:
                with tc.For_i(0, repeat):
                    body()
            else:
                body()

            nc.sync.dma_start(y_d[:], vf_out[:])

    nc.compile()
    return nc


def _prep_inputs(x, weight):
    """Host-side packing: one [128, BLOB] bf16 blob per core."""
    import ml_dtypes

    bf = ml_dtypes.bfloat16
    x = np.asarray(x, dtype=np.float32)
    w0 = np.asarray(weight, dtype=np.float32)[0]        # [O, I, DO, DI]

    # weights, dense per quad: W[(s,i,k) part, g, (o,d)]
    wp = w0.reshape(O, G, 4, 4, DO, DI).transpose(1, 2, 3, 5, 0, 4)
    wp = wp.reshape(G, 128, OD).transpose(1, 0, 2).reshape(128, W_SZ)
    wp = np.ascontiguousarray(wp.astype(bf))

    selm = np.zeros((128, 32), dtype=np.float32)
    for i in range(4):
        selm[32 * i:32 * i + 32] = np.eye(32, dtype=np.float32)
    repm = np.zeros((128, 128), dtype=np.float32)
    for i in range(4):
        repm[0:32, 32 * i:32 * i + 32] = np.eye(32, dtype=np.float32)
    selm = selm.astype(bf)
    repm = repm.astype(bf)

    in_maps = []
    for c in range(N_CORES):
        xs = x[BC * c:BC * (c + 1)]                     # [32, 1152, 8]
        xr = xs.reshape(BC, G, 4, 4, DI).transpose(1, 2, 3, 4, 0)
        # xr: [g, s, i, k, b]
        xtp = xr.reshape(G, 128, BC).transpose(1, 0, 2).reshape(128, XT_SZ)
        blob = np.empty((128, BLOB), dtype=bf)
        blob[:, XT_OFF:XT_OFF + XT_SZ] = xtp.astype(bf)
        blob[:, W_OFF:W_OFF + W_SZ] = wp
        blob[:, SEL_OFF:SEL_OFF + 32] = selm
        blob[:, REP_OFF:REP_OFF + 128] = repm
        in_maps.append({"blob": blob})
    return in_maps


def kernel(x, weight):
    from concourse.bass_utils import run_bass_kernel_spmd

    if "nc" not in _cached:
        _cached["nc"] = _build()
    in_maps = _prep_inputs(x, weight)
    res = run_bass_kernel_spmd(
        _cached["nc"], in_maps, core_ids=list(range(N_CORES)))
    return np.concatenate(
        [res.results[c]["y"] for c in range(N_CORES)], axis=0
    ).astype(np.float32)

